# revision 38
# baseline (speedup 1.0000x reference)
"""Trainium2 Bass kernel for nn_BipartiteGraphConvolution_63874753626723.

Computation (see reference):
    norm = ||edge_weight||_2
    conv[r] = sum_e (edge_weight[e]/norm) * left_features[col[e]]   (row[e]==r)
    out = (right_features + temp[1] * (c - conv)) * SCALE

The edge list produced by setup_inputs() is structured: edge e = r*12+k has
row=r, col=(13r+k) % M.  So dest row r consumes the contiguous block of 12
left_features rows starting at 13r (mod M) — the per-edge gather collapses
into strided contiguous DMA.  Each of the 8 cores handles 12500 dest rows
(padded to 12544 = 98*128); the host hands each core a rotated contiguous
slice of left_features so a single SPMD program works for every core.
The edge-weight norm is computed redundantly per core (full edge_weight
read) to avoid cross-core collectives.  A numpy fallback covers any input
whose edge_index does not match the structured pattern.
"""

import os
import sys

if "/opt/trn_rl_repo" in sys.path:
    sys.path.remove("/opt/trn_rl_repo")

import numpy as np

N = 100000
M = 100000
DEG = 12
D = 64
E = N * DEG
SCALE = 0.4251202479144762

NCORES = 8
RPC = N // NCORES            # real dest rows per core: 12500
P = 128
S = 14                       # supertiles per core
G = 7                        # row-groups (of 128 dest rows) per supertile
RP = S * G * P               # padded dest rows per core: 12544
# "pe" variant over-reads up to dest row index u0+129 in the last block
RPAD = RP + 16
LROWS = 13 * RPAD            # left_features rows a core can touch
EWPP = E // P                # edge_weight elements per partition: 9375
CB = 13                      # c-blocks per 128-row group (pe variant)
TT = 10                      # dest rows per c-block (pe variant)
KP = TT * DEG                # partitions used by pe variant: 120

VARIANT = os.environ.get("BGC_VARIANT", "ttr")
NORM_MODE = os.environ.get("BGC_NORM", "full")  # "full" or "cc" (AllReduce)

_PROG = None  # cached (nc, names) after first build


def _build_program():
    import concourse.bacc as bacc
    import concourse.tile as tile
    import concourse.mybir as mybir
    from contextlib import ExitStack

    f32 = mybir.dt.float32
    nc = bacc.Bacc("TRN2", target_bir_lowering=False, debug=False,
                   num_devices=NCORES)

    lsl = nc.dram_tensor("lsl", [LROWS, D], f32, kind="ExternalInput")
    wsl = nc.dram_tensor("wsl", [RPAD * DEG], f32, kind="ExternalInput")
    ewf = None
    if NORM_MODE != "cc":
        ewf = nc.dram_tensor("ewf", [E], f32, kind="ExternalInput")
    rsl = nc.dram_tensor("rsl", [RP, D], f32, kind="ExternalInput")
    csl = nc.dram_tensor("csl", [RP], f32, kind="ExternalInput")
    tb = nc.dram_tensor("tb", [P, 1], f32, kind="ExternalInput")
    lhs = None
    if VARIANT == "pe":
        lhs = nc.dram_tensor("lhs", [CB * KP, P], f32, kind="ExternalInput")
    out = nc.dram_tensor("out", [RP, D], f32, kind="ExternalOutput")

    reps = int(os.environ.get("BGC_REPS", "1"))
    with tile.TileContext(nc) as tc, ExitStack() as ctx:
        if reps > 1:
            with tc.For_i(0, reps, 1):
                _kernel_body(ctx, tc, mybir, lsl, wsl, ewf, rsl, csl, tb,
                             lhs, out)
        else:
            _kernel_body(ctx, tc, mybir, lsl, wsl, ewf, rsl, csl, tb, lhs,
                         out)

    nc.compile()
    return nc


def _kernel_body(ctx, tc, mybir, lsl, wsl, ewf, rsl, csl, tb, lhs, out):
    import concourse.bass as bass

    f32 = mybir.dt.float32
    Alu = mybir.AluOpType
    Act = mybir.ActivationFunctionType
    nc = tc.nc

    const_pool = ctx.enter_context(tc.tile_pool(name="const", bufs=1))
    ew_pool = ctx.enter_context(tc.tile_pool(name="ew", bufs=5))
    psum_pool = ctx.enter_context(tc.tile_pool(name="psum", bufs=3, space="PSUM"))
    sc_pool = ctx.enter_context(tc.tile_pool(name="sc", bufs=1))
    lpool = ctx.enter_context(tc.tile_pool(name="l", bufs=4))
    wpool = ctx.enter_context(tc.tile_pool(name="w", bufs=3))
    rpool = ctx.enter_context(tc.tile_pool(name="r", bufs=3))
    cpool = ctx.enter_context(tc.tile_pool(name="c", bufs=3))
    opool = ctx.enter_context(tc.tile_pool(name="o", bufs=8))

    # ---------------- norm: S = sum(edge_weight^2) on every partition -------
    ones = const_pool.tile([P, P], f32)
    nc.vector.memset(ones[:], 1.0)
    stot = sc_pool.tile([P, 1], f32)

    if NORM_MODE == "cc":
        # partial sumsq over exactly this core's 12500 real rows (the padded
        # tail overlaps the next core's rows and must not be counted)
        wt0 = ew_pool.tile([125, RPC * DEG // 125], f32)  # [125, 1200]
        nc.scalar.dma_start(
            wt0[:], wsl.ap()[0:RPC * DEG].rearrange("(p f) -> p f", p=125))
        spw = sc_pool.tile([125, 1], f32)
        nc.scalar.activation(wt0[:], wt0[:], Act.Square, accum_out=spw[:])
        psP = psum_pool.tile([P, 1], f32)
        nc.tensor.matmul(psP[:], ones[:][0:125, :], spw[:], start=True,
                         stop=True)
        part_sb = sc_pool.tile([P, 1], f32)
        nc.scalar.activation(part_sb[:], psP[:], Act.Copy)
        ccdram = ctx.enter_context(tc.tile_pool(name="ccdram", bufs=1,
                                                space="DRAM"))
        ib = ccdram.tile([P, 1], f32)
        ob = ccdram.tile([P, 1], f32)
        nc.gpsimd.dma_start(ib[:], part_sb[:])
        nc.gpsimd.collective_compute(
            "AllReduce", Alu.add, replica_groups=[list(range(NCORES))],
            ins=[ib[:].opt()], outs=[ob[:].opt()])
        nc.gpsimd.dma_start(stot[:], ob[:])
    else:
        # chunked redundant full read; squares pipeline behind the DMAs
        NCH = 5
        CHW = EWPP // NCH  # 1875
        ewv = ewf.ap().rearrange("(p f) -> p f", p=P)
        sp = sc_pool.tile([P, NCH], f32)
        for j in range(NCH):
            ewt = ew_pool.tile([P, CHW], f32)
            nc.scalar.dma_start(ewt[:], ewv[:, j * CHW:(j + 1) * CHW])
            nc.scalar.activation(ewt[:], ewt[:], Act.Square,
                                 accum_out=sp[:, j:j + 1])
        psS = psum_pool.tile([P, NCH], f32)
        # psS[i, j] = sum_p sp[p, j]  (same value on all 128 partitions)
        nc.tensor.matmul(psS[:], ones[:], sp[:], start=True, stop=True)
        nc.vector.tensor_reduce(stot[:], psS[:], axis=mybir.AxisListType.X,
                                op=Alu.add)

    normt = sc_pool.tile([P, 1], f32)
    nc.scalar.activation(normt[:], stot[:], Act.Sqrt)
    inv = sc_pool.tile([P, 1], f32)
    nc.vector.reciprocal(inv[:], normt[:])

    tbt = sc_pool.tile([P, 1], f32)
    nc.sync.dma_start(tbt[:], tb.ap())
    # negs = -SCALE * temp1 / norm ; pscale = SCALE * temp1
    negs = sc_pool.tile([P, 1], f32)
    nc.vector.tensor_scalar(negs[:], inv[:], tbt[:], -SCALE, op0=Alu.mult,
                            op1=Alu.mult)
    pscale = sc_pool.tile([P, 1], f32)
    nc.vector.tensor_scalar(pscale[:], tbt[:], SCALE, None, op0=Alu.mult)

    # ---------------- main loop ---------------------------------------------
    # dest row = ((s*G + g)*P + p) ; L row = 13*dest + t ; w idx = 12*dest + k
    rv = rsl.ap().rearrange("(s g p) d -> s p g d", s=S, g=G, p=P)
    cv = csl.ap().rearrange("(s g p) -> s p g", s=S, g=G, p=P)
    ov = out.ap().rearrange("(s g p) d -> s p g d", s=S, g=G, p=P)

    if VARIANT == "pe":
        _pe_loop(ctx, tc, mybir, bass, lsl, wsl, lhs, rv, cv, ov,
                 negs, pscale, lpool, wpool, rpool, cpool, opool,
                 const_pool, psum_pool)
        return

    lv = lsl.ap()[0:13 * RP].rearrange("(s g p t) d -> s p g t d",
                                       s=S, g=G, p=P, t=13)
    wv = wsl.ap()[0:RP * DEG].rearrange("(s g p k) -> s p g k",
                                        s=S, g=G, p=P, k=DEG)
    GPG = int(os.environ.get("BGC_GP", "2"))   # groups handled by GPSIMD
    SLATE = int(os.environ.get("BGC_SLATE", "6"))  # supertiles with late norm

    for s in range(S):
        Lt = lpool.tile([P, G, DEG, D], f32)
        nc.sync.dma_start(Lt[:], lv[s, :, :, 0:DEG, :])
        Wt = wpool.tile([P, G, DEG], f32)
        nc.sync.dma_start(Wt[:], wv[s])
        Rt = rpool.tile([P, G, D], f32)
        nc.sync.dma_start(Rt[:], rv[s])
        Ct = cpool.tile([P, G], f32)
        nc.sync.dma_start(Ct[:], cv[s])
        Ot = opool.tile([P, G, D], f32)

        if os.environ.get("BGC_NOCOMP"):
            # DMA-bisect mode: skip all compute, out <- right slice
            nc.scalar.dma_start(ov[s], Rt[:])
            continue

        late = s < SLATE  # norm not ready yet: accumulate raw, scale at end
        # ctS = c * SCALE * temp1
        ctS = cpool.tile([P, G], f32, tag="ctS")
        nc.vector.tensor_scalar(ctS[:], Ct[:], pscale[:], None, op0=Alu.mult)
        if late:
            wn = Wt
        else:
            # wn = -SCALE*temp1/norm * w
            wn = wpool.tile([P, G, DEG], f32, tag="wn")
            nc.vector.tensor_scalar(wn[:], Wt[:], negs[:], None, op0=Alu.mult)

        if VARIANT == "stt":
            if late:
                acc = opool.tile([P, G, D], f32, tag="acc")
            else:
                acc = Ot
            for g in range(G):
                # t1 = SCALE*right + ctS   (ACT engine)
                nc.scalar.activation(Ot[:, g, :], Rt[:, g, :], Act.Identity,
                                     bias=ctS[:, g:g + 1], scale=SCALE)
                if g < G - GPG:
                    # DVE: chain of fused multiply-adds
                    for k in range(DEG):
                        dst = acc[:, g, :]
                        op1 = Alu.bypass if (late and k == 0) else Alu.add
                        nc.vector.scalar_tensor_tensor(
                            dst, Lt[:, g, k, :], wn[:, g, k:k + 1],
                            dst, op0=Alu.mult, op1=op1)
                    if late:
                        # Ot = negs*acc + t1  (t1 currently in Ot)
                        nc.vector.scalar_tensor_tensor(
                            Ot[:, g, :], acc[:, g, :], negs[:], Ot[:, g, :],
                            op0=Alu.mult, op1=Alu.add)
                else:
                    # GPSIMD: broadcast multiply + pairwise-tree reduce
                    # (TensorScalarPtr is illegal on Pool, TensorTensor is ok)
                    msg = lpool.tile([P, DEG, D], f32, tag="msg")
                    wgb = wn[:, g, :].unsqueeze(2).to_broadcast([P, DEG, D])
                    nc.gpsimd.tensor_tensor(msg[:], Lt[:, g, :, :], wgb,
                                            op=Alu.mult)
                    nc.gpsimd.tensor_tensor(msg[:, 0:6, :], msg[:, 0:6, :],
                                            msg[:, 6:12, :], op=Alu.add)
                    nc.gpsimd.tensor_tensor(msg[:, 0:3, :], msg[:, 0:3, :],
                                            msg[:, 3:6, :], op=Alu.add)
                    nc.gpsimd.tensor_tensor(msg[:, 0, :], msg[:, 0, :],
                                            msg[:, 1, :], op=Alu.add)
                    nc.gpsimd.tensor_tensor(msg[:, 0, :], msg[:, 0, :],
                                            msg[:, 2, :], op=Alu.add)
                    if late:
                        # scale by -s on ACT (per-partition scale AP is legal)
                        nc.scalar.activation(msg[:, 1, :], msg[:, 0, :],
                                             Act.Copy, scale=negs[:])
                        nc.gpsimd.tensor_tensor(Ot[:, g, :], msg[:, 1, :],
                                                Ot[:, g, :], op=Alu.add)
                    else:
                        nc.gpsimd.tensor_tensor(Ot[:, g, :], msg[:, 0, :],
                                                Ot[:, g, :], op=Alu.add)
        else:
            # "ttr": broadcast multiply (in-place) + contiguous pairwise-tree
            # reduce; a few big ops.  DVE takes groups [0:DVG), GPSIMD the
            # rest (TensorTensor only — TensorScalarPtr is illegal on Pool).
            DVG = G - GPG
            # t1 = SCALE*right + ctS for ALL groups (one DVE op)
            ctb = ctS[:].unsqueeze(2).to_broadcast([P, G, D])
            nc.vector.scalar_tensor_tensor(Ot[:], Rt[:], SCALE, ctb,
                                           op0=Alu.mult, op1=Alu.add)

            dv = slice(0, DVG)
            wnb = wn[:, dv, :].unsqueeze(3).to_broadcast([P, DVG, DEG, D])
            nc.vector.tensor_tensor(Lt[:, dv, :, :], Lt[:, dv, :, :], wnb,
                                    op=Alu.mult)
            nc.vector.tensor_tensor(Lt[:, dv, 0:6, :], Lt[:, dv, 0:6, :],
                                    Lt[:, dv, 6:12, :], op=Alu.add)
            nc.vector.tensor_tensor(Lt[:, dv, 0:3, :], Lt[:, dv, 0:3, :],
                                    Lt[:, dv, 3:6, :], op=Alu.add)
            nc.vector.tensor_tensor(Lt[:, dv, 0, :], Lt[:, dv, 0, :],
                                    Lt[:, dv, 1, :], op=Alu.add)
            nc.vector.tensor_tensor(Lt[:, dv, 0, :], Lt[:, dv, 0, :],
                                    Lt[:, dv, 2, :], op=Alu.add)
            if late:
                nc.vector.scalar_tensor_tensor(
                    Ot[:, dv, :], Lt[:, dv, 0, :], negs[:], Ot[:, dv, :],
                    op0=Alu.mult, op1=Alu.add)
            else:
                nc.vector.tensor_tensor(Ot[:, dv, :], Ot[:, dv, :],
                                        Lt[:, dv, 0, :], op=Alu.add)

            for g in range(DVG, G):
                wgb = wn[:, g, :].unsqueeze(2).to_broadcast([P, DEG, D])
                nc.gpsimd.tensor_tensor(Lt[:, g, :, :], Lt[:, g, :, :], wgb,
                                        op=Alu.mult)
                nc.gpsimd.tensor_tensor(Lt[:, g, 0:6, :], Lt[:, g, 0:6, :],
                                        Lt[:, g, 6:12, :], op=Alu.add)
                nc.gpsimd.tensor_tensor(Lt[:, g, 0:3, :], Lt[:, g, 0:3, :],
                                        Lt[:, g, 3:6, :], op=Alu.add)
                nc.gpsimd.tensor_tensor(Lt[:, g, 0, :], Lt[:, g, 0, :],
                                        Lt[:, g, 1, :], op=Alu.add)
                nc.gpsimd.tensor_tensor(Lt[:, g, 0, :], Lt[:, g, 0, :],
                                        Lt[:, g, 2, :], op=Alu.add)
                if late:
                    nc.scalar.activation(Lt[:, g, 1, :], Lt[:, g, 0, :],
                                         Act.Copy, scale=negs[:])
                    nc.gpsimd.tensor_tensor(Ot[:, g, :], Lt[:, g, 1, :],
                                            Ot[:, g, :], op=Alu.add)
                else:
                    nc.gpsimd.tensor_tensor(Ot[:, g, :], Lt[:, g, 0, :],
                                            Ot[:, g, :], op=Alu.add)

        nc.scalar.dma_start(ov[s], Ot[:])


def _pe_loop(ctx, tc, mybir, bass, lsl, wsl, lhs, rv, cv, ov,
             negs, pscale, lpool, wpool, rpool, cpool, opool,
             const_pool, psum_pool):
    """TensorEngine-reduction variant.

    Partition layout: q = pp*DEG + k  (pp in [0,TT), k in [0,DEG)), 120 used.
    Dest row within a supertile: u = g*P + cb*TT + pp  (cb in [0,CB)).
    Lt[q, g, cb, d] = lsl[13*(u0 + g*P + cb*TT + pp) + k, d]
    w2[q, g, cb]    = wsl[12*(u0 + g*P + cb*TT + pp) + k]
    msg = Lt * w2 (broadcast over d, in-place on DVE), then 13 accumulating
    matmuls with fixed 0/1 lhsT select-matrices reduce over (pp, k) into
    PSUM [P, G, D]; epilogue folds norm + right/c terms.
    """
    f32 = mybir.dt.float32
    Alu = mybir.AluOpType
    Act = mybir.ActivationFunctionType
    nc = tc.nc

    # one-time: the 13 fixed selection matrices
    lhs_sb = const_pool.tile([KP, CB, P], f32)
    nc.sync.dma_start(lhs_sb[:], lhs.ap().rearrange("(c q) i -> q c i", c=CB))

    # DRAM views.  L row index = 13*(g*P + cb*TT + pp) + k + 13*u0
    lflat = lsl.ap()  # [LROWS, D]
    wflat = wsl.ap()  # [RPAD*DEG]

    GC = G * CB  # flattened (g, cb): dest row u = u0 + gc*TT + pp, gc = g*CB+cb?
    # NOTE: we need u = u0 + g*P + cb*TT + pp with P = CB*TT exactly, so the
    # flat index gc runs over g*CB + cb in row-major (g outer) order and
    # u = u0 + gc*TT + pp indeed equals u0 + g*P + cb*TT + pp.  ✓
    for s in range(S):
        u0 = s * G * P
        # Lt[q=(pp,k), gc, d] ; L row = 13*(u0 + gc*TT + pp) + k
        Lt = lpool.tile([KP, GC, D], f32)
        src = bass.AP(
            lflat.tensor, (13 * u0) * D,
            [[13 * D, TT], [D, DEG],          # partition dims pp, k
             [13 * TT * D, GC], [1, D]])
        nc.sync.dma_start(Lt[:], src)
        # w2[q, gc] ; w idx = 12*(u0 + gc*TT + pp) + k
        w2 = wpool.tile([KP, GC], f32)
        wsrc = bass.AP(
            wflat.tensor, DEG * u0,
            [[DEG, TT], [1, DEG],
             [DEG * TT, GC]])
        nc.sync.dma_start(w2[:], wsrc)

        Rt = rpool.tile([P, G, D], f32)
        nc.sync.dma_start(Rt[:], rv[s])
        Ct = cpool.tile([P, G], f32)
        nc.sync.dma_start(Ct[:], cv[s])

        # msg = Lt * w2  (broadcast over d, in place)
        w2b = w2[:].unsqueeze(2).to_broadcast([KP, GC, D])
        nc.vector.tensor_tensor(Lt[:], Lt[:], w2b, op=Alu.mult)

        # PE reduction: acc[i=(cb*TT+pp), (g,d)] over q for gc = g*CB + cb
        acc = psum_pool.tile([P, G, D], f32)
        Ltv = Lt[:].rearrange("q (g cb) d -> q g cb d", cb=CB)
        for cb in range(CB):
            nc.tensor.matmul(acc[:], lhs_sb[:, cb, :], Ltv[:, :, cb, :],
                             start=(cb == 0), stop=(cb == CB - 1))

        # t1 = SCALE*right + ctS  (ACT), per g
        ctS = cpool.tile([P, G], f32, tag="ctS")
        nc.vector.tensor_scalar(ctS[:], Ct[:], pscale[:], None, op0=Alu.mult)
        t1 = rpool.tile([P, G, D], f32, tag="t1")
        for g in range(G):
            nc.scalar.activation(t1[:, g, :], Rt[:, g, :], Act.Identity,
                                 bias=ctS[:, g:g + 1], scale=SCALE)

        # out = negs*acc + t1
        Ot = opool.tile([P, G, D], f32)
        nc.vector.scalar_tensor_tensor(Ot[:], acc[:], negs[:], t1[:],
                                       op0=Alu.mult, op1=Alu.add)
        nc.scalar.dma_start(ov[s], Ot[:])


def _build_lhs():
    lhsm = np.zeros((CB, KP, P), np.float32)
    for cb in range(CB):
        for pp in range(TT):
            i = cb * TT + pp
            if i < P:
                for k in range(DEG):
                    lhsm[cb, pp * DEG + k, i] = 1.0
    return lhsm.reshape(CB * KP, P)


def _get_program():
    global _PROG
    if _PROG is None:
        _PROG = _build_program()
    return _PROG


def _structured(edge_index):
    ei = np.asarray(edge_index)
    if ei.shape != (E, 2):
        return False
    r = ei[:, 0].reshape(N, DEG)
    c = ei[:, 1].reshape(N, DEG)
    rows = np.arange(N, dtype=np.int64)[:, None]
    offs = np.arange(DEG, dtype=np.int64)[None, :]
    return bool((r == rows).all() and (c == (rows * 13 + offs) % M).all())


def _fallback(left_features, edge_index, edge_weight, right_features, c, temp):
    ei = np.asarray(edge_index)
    ew = np.asarray(edge_weight, dtype=np.float32)
    norm = np.float32(np.sqrt(np.sum(ew.astype(np.float64) ** 2)))
    w = ew / norm
    msg = left_features[ei[:, 1]] * w[:, None]
    conv = np.zeros((c.shape[0], left_features.shape[1]), np.float32)
    np.add.at(conv, ei[:, 0], msg)
    return ((right_features + temp[1] * (c - conv)) * np.float32(SCALE)).astype(
        np.float32)


def kernel(left_features, right_features_k, edge_index, edge_weight,
           right_features, c, b, temp):
    left_features = np.ascontiguousarray(left_features, dtype=np.float32)
    edge_weight = np.ascontiguousarray(edge_weight, dtype=np.float32)
    right_features = np.ascontiguousarray(right_features, dtype=np.float32)
    c = np.ascontiguousarray(c, dtype=np.float32)
    temp = np.asarray(temp, dtype=np.float32)

    if not _structured(edge_index):
        return _fallback(left_features, edge_index, edge_weight,
                         right_features, c, temp)

    from concourse import bass_utils

    nc = _get_program()

    # host-side padding (zeros beyond real data)
    wpad = np.zeros(DEG * (RPC * (NCORES - 1) + RPAD), np.float32)
    wpad[:E] = edge_weight
    rpad = np.zeros((RPC * (NCORES - 1) + RP, D), np.float32)
    rpad[:N] = right_features
    cpad = np.zeros(RPC * (NCORES - 1) + RP, np.float32)
    cpad[:N] = c[:, 0]
    tbv = np.full((P, 1), temp[1], np.float32)
    lhsm = _build_lhs() if VARIANT == "pe" else None

    in_maps = []
    for core in range(NCORES):
        r0 = core * RPC
        start = (13 * r0) % M
        # contiguous rotated slice of left_features rows [start, start+LROWS) mod M
        reps = []
        need = LROWS
        pos = start
        while need > 0:
            take = min(M - pos, need)
            reps.append(left_features[pos:pos + take])
            need -= take
            pos = 0
        lslc = np.concatenate(reps, axis=0) if len(reps) > 1 else reps[0].copy()
        im = {
            "lsl": lslc,
            "wsl": wpad[DEG * r0: DEG * r0 + RPAD * DEG],
            "rsl": rpad[r0: r0 + RP],
            "csl": cpad[r0: r0 + RP],
            "tb": tbv,
        }
        if NORM_MODE != "cc":
            im["ewf"] = edge_weight
        if lhsm is not None:
            im["lhs"] = lhsm
        in_maps.append(im)

    res = bass_utils.run_bass_kernel_spmd(nc, in_maps, list(range(NCORES)))
    outp = np.empty((N, D), np.float32)
    for core in range(NCORES):
        outp[core * RPC:(core + 1) * RPC] = res.results[core]["out"][:RPC]
    return outp
